# revision 1
# baseline (speedup 1.0000x reference)
"""GCN (2-layer, DGL GraphConv norm='both') on 8 Trainium2 NeuronCores.

Strategy (dst-sharded graph parallelism):
  - Nodes are split into 8 contiguous ranges of 12500; core c owns the
    destination nodes [c*12500, (c+1)*12500) and every edge whose dst lands
    there.  Host preprocessing groups each core's edges by 128-wide dst
    block and 32k-node source slab (dma_gather uses int16 indices), pads
    each (block, slab) run to whole 128-edge chunks (uniform across cores
    so the SPMD program is identical), and emits flat index/weight arrays
    in device consumption order.
  - Per 128-edge chunk the device: (1) dma_gathers the 128 source rows,
    (2) builds a weighted one-hot mask M[e, d] = (dst_e == d) * w_e on the
    vector engine (w folds both degree norms), and (3) runs
    gathered.T @ M on the tensor engine, accumulating the block aggregate
    in PSUM.  Blocks are processed in groups of 8 sharing packed PSUM
    accumulators.  Aggregation, W1, relu, W2 are fused per block; the
    projected features are all-gathered across cores and the same
    machinery (same indices/masks) runs layer 2.

`kernel(**inputs)` takes the full-size numpy inputs and returns the full
[100000, 40] output; sharding/compilation happens inside.
"""

import numpy as np

import concourse.bass as bass
import concourse.bacc as bacc
import concourse.tile as tile
import concourse.mybir as mybir

F32 = mybir.dt.float32
I16 = mybir.dt.int16

NCORES = 8
GBLK = 8          # dst blocks per PSUM group
MAX_SLAB = 32767  # dma_gather int16 index limit


# --------------------------------------------------------------------------
# Host-side graph preprocessing
# --------------------------------------------------------------------------

def preprocess(src, dst, n_nodes):
    """Partition + schedule. Returns (cfg, per_core dict arrays)."""
    n = n_nodes
    npc = n // NCORES
    nblk = (npc + 127) // 128
    nslab = max(1, -(-n // MAX_SLAB))
    slab_rows = -(-n // nslab)

    ones = np.ones(src.shape[0], np.float64)
    deg_out = np.clip(np.bincount(src, weights=ones, minlength=n), 1.0, None)
    deg_in = np.clip(np.bincount(dst, weights=ones, minlength=n), 1.0, None)
    w_edge = (deg_out[src] ** -0.5 * deg_in[dst] ** -0.5).astype(np.float32)

    # per-core edge sets, sorted by (block, slab)
    owner = dst // npc
    cores = []
    counts = np.zeros((NCORES, nblk, nslab), np.int64)
    for c in range(NCORES):
        sel = np.where(owner == c)[0]
        dl = dst[sel] - c * npc
        blk = dl // 128
        slab = src[sel] // slab_rows
        o = np.lexsort((slab, blk))
        sel = sel[o]
        cores.append((src[sel], (dl[o]).astype(np.int64), w_edge[sel],
                      blk[o], slab[o]))
        np.add.at(counts[c], (blk[o], slab[o]), 1)

    c_bs = -(-counts.max(axis=0) // 128)          # [nblk, nslab] chunks
    k_blk = c_bs.sum(axis=1)                      # chunks per block
    assert (k_blk > 0).all(), "empty dst block not supported"

    # schedule: groups of GBLK blocks; within a group: slab-major
    groups = [(g0, min(g0 + GBLK, nblk)) for g0 in range(0, nblk, GBLK)]
    chunk_meta = []     # per chunk: (block, kk_in_block, q)
    windows = []        # per window: (slab, q0, nchunks)
    base_q = np.zeros((nblk, nslab), np.int64)
    kk_ctr = np.zeros(nblk, np.int64)
    q = 0
    for (g0, g1) in groups:
        for s in range(nslab):
            nch = int(c_bs[g0:g1, s].sum())
            if nch == 0:
                continue
            windows.append((s, q, nch))
            for b in range(g0, g1):
                base_q[b, s] = q
                for _ in range(int(c_bs[b, s])):
                    chunk_meta.append((b, int(kk_ctr[b]), q))
                    kk_ctr[b] += 1
                    q += 1
    nchunk = q
    assert (kk_ctr == k_blk).all()

    # per-core slot arrays
    per_core = []
    for c in range(NCORES):
        cs, cdl, cw, cblk, cslab = cores[c]
        idx_cl = np.zeros((nchunk, 128), np.int16)
        dst_cl = np.full((nchunk, 128), -1.0, np.float32)
        w_cl = np.zeros((nchunk, 128), np.float32)
        # position of each edge within its (b, s) run
        run_id = cblk * nslab + cslab
        pos = np.arange(len(cs)) - np.concatenate(
            [[0], np.cumsum(np.bincount(run_id, minlength=nblk * nslab))]
        )[run_id]
        ch = base_q[cblk, cslab] + pos // 128
        ln = pos % 128
        idx_cl[ch, ln] = (cs - cslab * slab_rows).astype(np.int16)
        dst_cl[ch, ln] = (cdl - cblk * 128).astype(np.float32)
        w_cl[ch, ln] = cw
        # wrap indices into the [16, n/16] layout, replicated to 128 parts
        flat = idx_cl.reshape(-1)                     # chunk-major
        wrapped = flat.reshape(-1, 16).T              # [16, nchunk*8]
        idx_all = np.tile(wrapped, (8, 1))            # [128, nchunk*8]
        per_core.append({
            "idx": np.ascontiguousarray(idx_all),
            "dcols": np.ascontiguousarray(dst_cl.T),  # [128, nchunk]
            "wcols": np.ascontiguousarray(w_cl.T),
        })

    cfg = dict(n=n, npc=npc, nblk=nblk, nslab=nslab, slab_rows=slab_rows,
               nchunk=nchunk, windows=windows,
               chunk_meta=chunk_meta, k_blk=[int(v) for v in k_blk])
    return cfg, per_core


# --------------------------------------------------------------------------
# Device program
# --------------------------------------------------------------------------

def build_program(cfg, feat, hid, cls, debug_p=False):
    n, npc, nblk = cfg["n"], cfg["npc"], cfg["nblk"]
    nslab, slab_rows = cfg["nslab"], cfg["slab_rows"]
    nchunk = cfg["nchunk"]
    windows, chunk_meta = cfg["windows"], cfg["chunk_meta"]
    k_blk = cfg["k_blk"]
    clsp = -(-cls * 4 // 256) * 64          # pad cls rows to 256B

    nc = bacc.Bacc("TRN2", target_bir_lowering=False, debug=False,
                   num_devices=NCORES)

    xin = nc.dram_tensor("xin", [n, feat], F32, kind="ExternalInput")
    w1 = nc.dram_tensor("w1", [feat, hid], F32, kind="ExternalInput")
    b1 = nc.dram_tensor("b1", [hid], F32, kind="ExternalInput")
    w2 = nc.dram_tensor("w2", [hid, clsp], F32, kind="ExternalInput")
    b2 = nc.dram_tensor("b2", [clsp], F32, kind="ExternalInput")
    idx = nc.dram_tensor("idx", [128, nchunk * 8], I16, kind="ExternalInput")
    dcols = nc.dram_tensor("dcols", [128, nchunk], F32, kind="ExternalInput")
    wcols = nc.dram_tensor("wcols", [128, nchunk], F32, kind="ExternalInput")
    iota = nc.dram_tensor("iota", [128, 128], F32, kind="ExternalInput")
    eye = nc.dram_tensor("eye", [clsp, clsp], F32, kind="ExternalInput")
    out = nc.dram_tensor("out", [npc, cls], F32, kind="ExternalOutput")

    p_slice = nc.dram_tensor("p_slice", [npc, clsp], F32)
    p_dbg = (nc.dram_tensor("p_dbg", [npc, clsp], F32,
                            kind="ExternalOutput") if debug_p else None)
    p_full = nc.dram_tensor("p_full", [NCORES * npc, clsp], F32,
                            addr_space="Shared")

    # map: block -> (start chunk meta)
    blk_last = {}
    for (b, kk, q) in chunk_meta:
        if kk == k_blk[b] - 1:
            blk_last[b] = q

    with tile.TileContext(nc) as tc:
        from concourse import library_config
        nc.gpsimd.load_library(library_config.mlp)
        with (
            tc.tile_pool(name="const", bufs=1) as cpool,
            tc.tile_pool(name="cols", bufs=1) as colpool,
        ):
            w1_t = cpool.tile([feat, hid], F32)
            w2_t = cpool.tile([hid, clsp], F32)
            b1_t = cpool.tile([hid, 1], F32)
            b2_t = cpool.tile([clsp, 1], F32)
            iota_t = cpool.tile([128, 128], F32)
            eye_t = cpool.tile([clsp, clsp], F32)
            nc.sync.dma_start(w1_t[:], w1[:])
            nc.sync.dma_start(w2_t[:], w2[:])
            nc.sync.dma_start(b1_t[:], b1[:])
            nc.sync.dma_start(b2_t[:], b2[:])
            nc.sync.dma_start(iota_t[:], iota[:])
            nc.sync.dma_start(eye_t[:], eye[:])

            dcol_t = colpool.tile([128, nchunk], F32)
            wcol_t = colpool.tile([128, nchunk], F32)
            idx_t = colpool.tile([128, nchunk * 8], I16)
            nc.sync.dma_start(dcol_t[:], dcols[:])
            nc.sync.dma_start(wcol_t[:], wcols[:])
            nc.sync.dma_start(idx_t[:], idx[:])

            max_win = max(nch for (_, _, nch) in windows)

            def layer(tagp, src_dram, elem, proj_out):
                """One aggregation layer; proj_out(b, agg_slice_ap)."""
                with (
                    tc.tile_pool(name=f"{tagp}g", bufs=2) as gpool,
                    tc.tile_pool(name=f"{tagp}mask", bufs=6) as mpool,
                    tc.tile_pool(name=f"{tagp}agg", bufs=2,
                                 space=bass.MemorySpace.PSUM) as apool,
                ):
                    # last chunk targeting each (group, bank): stop flag
                    # must close the whole psum bank, not a single block.
                    bank_last = {}
                    for (b2, kk2, q2) in chunk_meta:
                        bank_last[(b2 // GBLK, (b2 % GBLK) * 512 // 2048)] = q2
                    agg = None
                    agg_blk0 = -1
                    started_banks = set()
                    for (s, q0, nch) in windows:
                        g = gpool.tile([128, max_win, elem], F32, tag="g")
                        r0 = s * slab_rows
                        nc.gpsimd.dma_gather(
                            g[:, :nch, :],
                            src_dram[r0:min(r0 + slab_rows, n), :],
                            idx_t[:, q0 * 8:(q0 + nch) * 8],
                            num_idxs=nch * 128,
                            num_idxs_reg=nch * 128,
                            elem_size=elem,
                            single_packet=False,
                        )
                        for i in range(nch):
                            q = q0 + i
                            b, kk, _ = chunk_meta[q]
                            grp = b // GBLK
                            if grp != agg_blk0:
                                gw = min(GBLK, nblk - grp * GBLK)
                                agg = apool.tile([elem, gw * 128], F32,
                                                 tag="agg")
                                agg_blk0 = grp
                                started_banks = set()
                            bo = (b % GBLK) * 128
                            # matmul start=True resets the WHOLE psum bank,
                            # so only the first matmul into each bank of the
                            # group tile may set it (resets all its slices).
                            bank = (b % GBLK) * 128 * 4 // 2048
                            start = bank not in started_banks
                            started_banks.add(bank)
                            mask = mpool.tile([128, 128], F32, tag="mask")
                            nc.vector.tensor_scalar(
                                mask[:], iota_t[:],
                                dcol_t[:, q:q + 1], wcol_t[:, q:q + 1],
                                op0=mybir.AluOpType.is_equal,
                                op1=mybir.AluOpType.mult,
                            )
                            nc.tensor.matmul(
                                agg[:, bo:bo + 128], g[:, i, :], mask[:],
                                start=start,
                                stop=(q == bank_last[(grp, bank)]),
                                skip_group_check=True,
                            )
                            if kk == k_blk[b] - 1:
                                proj_out(b, agg[:, bo:bo + 128])

            with tc.tile_pool(name="l1s", bufs=3) as spool, \
                 tc.tile_pool(name="l1p", bufs=2,
                              space=bass.MemorySpace.PSUM) as mmpool, \
                 tc.tile_pool(name="l1q", bufs=1,
                              space=bass.MemorySpace.PSUM) as ppool, \
                 tc.tile_pool(name="l1t", bufs=1,
                              space=bass.MemorySpace.PSUM) as tpool:

                def l1_block(b, agg_ap):
                    rows = min(128, npc - b * 128)
                    aggs = spool.tile([feat, 128], F32, tag="aggs")
                    nc.scalar.copy(aggs[:], agg_ap)
                    mm1 = mmpool.tile([hid, 128], F32, tag="mm1")
                    nc.tensor.matmul(mm1[:], w1_t[:], aggs[:],
                                     start=True, stop=True)
                    h_t = spool.tile([hid, 128], F32, tag="h")
                    nc.scalar.activation(h_t[:], mm1[:],
                                         mybir.ActivationFunctionType.Relu,
                                         bias=b1_t[:])
                    p1 = ppool.tile([clsp, 128], F32, tag="p1")
                    nc.tensor.matmul(p1[:], w2_t[:], h_t[:],
                                     start=True, stop=True)
                    p1s = spool.tile([clsp, 128], F32, tag="p1s")
                    nc.scalar.copy(p1s[:], p1[:])
                    ptr = tpool.tile([128, clsp], F32, tag="ptr")
                    nc.tensor.transpose(ptr[:], p1s[:], eye_t[:])
                    pout = spool.tile([128, clsp], F32, tag="pout")
                    nc.scalar.copy(pout[:], ptr[:])
                    nc.sync.dma_start(
                        p_slice[b * 128:b * 128 + rows, :], pout[:rows, :])

                layer("l1", xin, feat, l1_block)

            nc.gpsimd.collective_compute(
                "AllGather",
                mybir.AluOpType.bypass,
                replica_groups=[list(range(NCORES))],
                ins=[p_slice[:]],
                outs=[p_full[:]],
            )

            if debug_p:
                nc.sync.dma_start(p_dbg[:], p_slice[:])

            with tc.tile_pool(name="l2s", bufs=3) as spool2, \
                 tc.tile_pool(name="l2t", bufs=2,
                              space=bass.MemorySpace.PSUM) as tpool2:

                def l2_block(b, agg_ap):
                    rows = min(128, npc - b * 128)
                    oS = spool2.tile([clsp, 128], F32, tag="oS")
                    nc.scalar.activation(oS[:], agg_ap,
                                         mybir.ActivationFunctionType.Identity,
                                         bias=b2_t[:])
                    oT = tpool2.tile([128, clsp], F32, tag="oT")
                    nc.tensor.transpose(oT[:], oS[:], eye_t[:])
                    oF = spool2.tile([128, clsp], F32, tag="oF")
                    nc.scalar.copy(oF[:], oT[:])
                    nc.sync.dma_start(
                        out[b * 128:b * 128 + rows, :], oF[:rows, :cls])

                layer("l2", p_full, clsp, l2_block)

    nc.compile()
    return nc


# --------------------------------------------------------------------------
# Input assembly + entry points
# --------------------------------------------------------------------------

def make_in_maps(cfg, per_core, x, W1, b1, W2, b2):
    cls = W2.shape[1]
    clsp = -(-cls * 4 // 256) * 64
    W2p = np.zeros((W2.shape[0], clsp), np.float32)
    W2p[:, :cls] = W2
    b2p = np.zeros(clsp, np.float32)
    b2p[:cls] = b2
    iota = np.broadcast_to(np.arange(128, dtype=np.float32), (128, 128)).copy()
    eye = np.eye(clsp, dtype=np.float32)
    ins = []
    for c in range(NCORES):
        pc = per_core[c]
        ins.append({
            "xin": x, "w1": W1, "b1": b1, "w2": W2p, "b2": b2p,
            "idx": pc["idx"], "dcols": pc["dcols"], "wcols": pc["wcols"],
            "iota": iota, "eye": eye,
        })
    return ins


_CACHE = {}


def _get_compiled(key, cfg, feat, hid, cls):
    if key not in _CACHE:
        _CACHE[key] = build_program(cfg, feat, hid, cls)
    return _CACHE[key]


class _Runner:
    """Persistent PJRT runner (mirrors bass2jax.run_bass_via_pjrt's
    multi-core path) so repeated executions reuse the loaded NEFF and
    device-resident inputs."""

    def __init__(self, nc, in_maps):
        import jax
        import concourse.mybir as mb
        from concourse.bass2jax import (
            _bass_exec_p, install_neuronx_cc_hook, partition_id_tensor,
            shard_map, Mesh, PartitionSpec,
        )

        install_neuronx_cc_hook()
        n_cores = len(in_maps)
        partition_name = (nc.partition_id_tensor.name
                          if nc.partition_id_tensor else None)
        in_names, out_names, out_avals, zero_outs = [], [], [], []
        for alloc in nc.m.functions[0].allocations:
            if not isinstance(alloc, mb.MemoryLocationSet):
                continue
            name = alloc.memorylocations[0].name
            if alloc.kind == "ExternalInput":
                if name != partition_name:
                    in_names.append(name)
            elif alloc.kind == "ExternalOutput":
                out_names.append(name)
                shape = tuple(alloc.tensor_shape)
                dtype = mb.dt.np(alloc.dtype)
                out_avals.append(jax.core.ShapedArray(shape, dtype))
                zero_outs.append(np.zeros(shape, dtype))
        n_params = len(in_names)
        all_names = in_names + out_names
        if partition_name is not None:
            all_names.append(partition_name)

        def _body(*args):
            operands = list(args)
            if partition_name is not None:
                operands.append(partition_id_tensor())
            return tuple(_bass_exec_p.bind(
                *operands,
                out_avals=tuple(out_avals),
                in_names=tuple(all_names),
                out_names=tuple(out_names),
                lowering_input_output_aliases=(),
                sim_require_finite=True,
                sim_require_nnan=True,
                nc=nc,
            ))

        devices = jax.devices()[:n_cores]
        mesh = Mesh(np.asarray(devices), ("core",))
        nin = n_params + len(out_names)
        self._fn = jax.jit(
            shard_map(_body, mesh=mesh,
                      in_specs=(PartitionSpec("core"),) * nin,
                      out_specs=(PartitionSpec("core"),) * len(out_names),
                      check_rep=False),
            keep_unused=True,
        )
        concat_in = [
            np.concatenate([np.asarray(in_maps[c][k]) for c in range(n_cores)],
                           axis=0)
            for k in in_names
        ]
        concat_zeros = [
            np.zeros((n_cores * z.shape[0], *z.shape[1:]), z.dtype)
            for z in zero_outs
        ]
        self._dev_in = [jax.device_put(a) for a in concat_in + concat_zeros]
        self.out_names = out_names
        self.out_avals = out_avals
        self.n_cores = n_cores

    def run(self):
        outs = self._fn(*self._dev_in)
        for o in outs:
            o.block_until_ready()
        return outs

    def results(self):
        outs = self.run()
        return [
            {
                name: np.asarray(outs[i]).reshape(
                    self.n_cores, *self.out_avals[i].shape)[c]
                for i, name in enumerate(self.out_names)
            }
            for c in range(self.n_cores)
        ]


_RUNNER = {}


def _get_runner(key, nc, ins):
    if key not in _RUNNER:
        _RUNNER[key] = _Runner(nc, ins)
    return _RUNNER[key]


def _prep_all(x, W1, b1, W2, b2, src, dst):
    x = np.asarray(x, np.float32)
    src = np.asarray(src, np.int32)
    dst = np.asarray(dst, np.int32)
    n, feat = x.shape
    hid = np.asarray(W1).shape[1]
    cls = np.asarray(W2).shape[1]
    cfg, per_core = preprocess(src, dst, n)
    key = (n, feat, hid, cls, cfg["nchunk"])
    nc = _get_compiled(key, cfg, feat, hid, cls)
    ins = make_in_maps(cfg, per_core, x, np.asarray(W1, np.float32),
                       np.asarray(b1, np.float32), np.asarray(W2, np.float32),
                       np.asarray(b2, np.float32))
    return key, cfg, nc, ins


def time_kernel(inputs, iters=10):
    """Wall-clock the on-device execution (inputs device-resident)."""
    import time
    key, cfg, nc, ins = _prep_all(**inputs)
    r = _get_runner(key, nc, ins)
    r.run()  # warm-up (triggers NEFF compile/load)
    times = []
    for _ in range(iters):
        t0 = time.perf_counter()
        r.run()
        times.append(time.perf_counter() - t0)
    times.sort()
    return times[len(times) // 4] * 1e9   # lower-quartile wall time


def kernel(x, W1, b1, W2, b2, src, dst):
    n = np.asarray(x).shape[0]
    key, cfg, nc, ins = _prep_all(x, W1, b1, W2, b2, src, dst)
    r = _get_runner(key, nc, ins)
    res = r.results()
    return np.concatenate([res[c]["out"] for c in range(NCORES)],
                          axis=0)[:n]



# revision 2
# speedup vs baseline: 1.0765x; 1.0765x over previous
"""GCN (2-layer, DGL GraphConv norm='both') on 8 Trainium2 NeuronCores.

Strategy (dst-sharded graph parallelism, slab-major bf16 gather):
  - Nodes split into 8 ranges of 12500; core c owns destination nodes
    [c*12500, (c+1)*12500) and every edge whose dst lands there.
  - Per core, edges are scheduled slab-major: for each source slab
    (4 x 25000 rows of x) and each destination group (4 blocks = 512 dst
    columns), the window's edges are sorted by src (monotone gather
    addresses) and padded to whole 128-edge chunks (uniform across cores
    so the SPMD program is identical).
  - Device per window: one SWDGE dma_gather fetches the window's source
    rows (bf16, 256B each); per 128-edge chunk the vector engine builds a
    [128, 512] one-hot mask M[e, d] = (dst_e == d) * w_e (w folds both
    degree norms) and the tensor engine accumulates gathered.T @ M into a
    group PSUM tile, flushed once per window into a full-core SBUF
    accumulator [128, 12800] (so each slab is gathered exactly once).
  - Layer 1 output is projected (W1, relu, W2) per 128-node block and
    stored as p = [n, 128] bf16 (cols 40+ zero); after an AllGather the
    same window/idx/mask arrays aggregate p for layer 2 (aggregate-after-
    project is exact since W2 is linear).

`kernel(**inputs)` takes full-size numpy inputs and returns the full
[100000, 40] output; sharding/compilation happens inside.
"""

import numpy as np
import ml_dtypes

import concourse.bass as bass
import concourse.bacc as bacc
import concourse.tile as tile
import concourse.mybir as mybir
from concourse import library_config

F32 = mybir.dt.float32
BF16 = mybir.dt.bfloat16
I16 = mybir.dt.int16

NCORES = 8
GBLK = 4
NSLAB = 4


# --------------------------------------------------------------------------
# Host-side graph preprocessing (index/schedule only — no feature data)
# --------------------------------------------------------------------------

def preprocess(src, dst, n):
    npc = n // NCORES
    nblk = -(-npc // 128)
    grp_cols = GBLK * 128
    ngrp = -(-nblk // GBLK)
    slab_rows = -(-n // NSLAB)

    ones = np.ones(src.shape[0], np.float64)
    deg_out = np.clip(np.bincount(src, weights=ones, minlength=n), 1.0, None)
    deg_in = np.clip(np.bincount(dst, weights=ones, minlength=n), 1.0, None)
    w_edge = (deg_out[src] ** -0.5 * deg_in[dst] ** -0.5).astype(np.float32)

    owner = dst // npc
    cores = []
    counts = np.zeros((NCORES, NSLAB, ngrp), np.int64)
    for c in range(NCORES):
        sel = np.where(owner == c)[0]
        cs = src[sel]
        dl = dst[sel] - c * npc
        s = cs // slab_rows
        g2 = dl // grp_cols
        o = np.lexsort((cs, g2, s))
        sel = sel[o]
        cores.append((cs[o], dl[o], w_edge[sel], s[o], g2[o]))
        np.add.at(counts[c], (s[o], g2[o]), 1)

    c_sg = np.maximum(-(-counts.max(axis=0) // 128), 1)

    windows = []
    q = 0
    for s in range(NSLAB):
        for g2 in range(ngrp):
            nch = int(c_sg[s, g2])
            windows.append((s, g2, q, nch))
            q += nch
    nchunk = q
    base_q = np.zeros((NSLAB, ngrp), np.int64)
    for (s, g2, q0, nch) in windows:
        base_q[s, g2] = q0

    per_core = []
    for c in range(NCORES):
        cs, cdl, cw, cslab, cgrp = cores[c]
        idx_fl = np.zeros(nchunk * 128, np.int16)
        dcol = np.full(nchunk * 128, -1.0, np.float32)
        wcol = np.zeros(nchunk * 128, np.float32)
        run_id = cslab * ngrp + cgrp
        run_start = np.concatenate(
            [[0], np.cumsum(np.bincount(run_id, minlength=NSLAB * ngrp))])
        pos = np.arange(len(cs)) - run_start[run_id]
        slot = base_q[cslab, cgrp] * 128 + pos
        idx_fl[slot] = (cs - cslab * slab_rows).astype(np.int16)
        dcol[slot] = (cdl - cgrp * grp_cols).astype(np.float32)
        wcol[slot] = cw
        wrapped = idx_fl.reshape(-1, 16).T
        per_core.append({
            "idx": np.ascontiguousarray(np.tile(wrapped, (8, 1))),
            "dcol": np.ascontiguousarray(dcol.reshape(-1, 128).T),
            "wcol": np.ascontiguousarray(wcol.reshape(-1, 128).T),
        })

    cfg = dict(n=n, npc=npc, nblk=nblk, grp_cols=grp_cols, ngrp=ngrp,
               slab_rows=slab_rows, nchunk=nchunk, windows=windows,
               max_ch=int(c_sg.max()))
    return cfg, per_core


# --------------------------------------------------------------------------
# Device program
# --------------------------------------------------------------------------

def build_program(cfg, feat, hid, cls):
    n, npc, nblk = cfg["n"], cfg["npc"], cfg["nblk"]
    grp_cols, ngrp = cfg["grp_cols"], cfg["ngrp"]
    slab_rows, nchunk = cfg["slab_rows"], cfg["nchunk"]
    windows, max_ch = cfg["windows"], cfg["max_ch"]
    ncols = ngrp * grp_cols
    clsp = 128   # p padded to 128 cols (bf16 -> 256B gather rows)

    nc = bacc.Bacc("TRN2", target_bir_lowering=False, debug=False,
                   num_devices=NCORES)

    xg = nc.dram_tensor("xg", [n, feat], BF16, kind="ExternalInput")
    w1 = nc.dram_tensor("w1", [feat, hid], BF16, kind="ExternalInput")
    b1 = nc.dram_tensor("b1", [hid], F32, kind="ExternalInput")
    w2 = nc.dram_tensor("w2", [hid, clsp], BF16, kind="ExternalInput")
    b2 = nc.dram_tensor("b2", [64], F32, kind="ExternalInput")
    idx = nc.dram_tensor("idx", [128, nchunk * 8], I16, kind="ExternalInput")
    dcols = nc.dram_tensor("dcols", [128, nchunk], F32, kind="ExternalInput")
    wcols = nc.dram_tensor("wcols", [128, nchunk], F32, kind="ExternalInput")
    iota = nc.dram_tensor("iota", [128, grp_cols], F32, kind="ExternalInput")
    eye1 = nc.dram_tensor("eye1", [clsp, clsp], BF16, kind="ExternalInput")
    eye2 = nc.dram_tensor("eye2", [64, 64], F32, kind="ExternalInput")
    out = nc.dram_tensor("out", [npc, cls], F32, kind="ExternalOutput")

    p_slice = nc.dram_tensor("p_slice", [npc, clsp], BF16)
    p_full = nc.dram_tensor("p_full", [NCORES * npc, clsp], BF16,
                            addr_space="Shared")

    with tile.TileContext(nc) as tc:
        nc.gpsimd.load_library(library_config.mlp)
        with tc.tile_pool(name="const", bufs=1) as cpool:
            w1_t = cpool.tile([feat, hid], BF16)
            w2_t = cpool.tile([hid, clsp], BF16)
            b1_t = cpool.tile([hid, 1], F32)
            b2_t = cpool.tile([64, 1], F32)
            iota_t = cpool.tile([128, grp_cols], F32)
            eye1_t = cpool.tile([clsp, clsp], BF16)
            eye2_t = cpool.tile([64, 64], F32)
            dcol_t = cpool.tile([128, nchunk], F32)
            wcol_t = cpool.tile([128, nchunk], F32)
            idx_t = cpool.tile([128, nchunk * 8], I16)
            for t, d in ((w1_t, w1), (w2_t, w2), (b1_t, b1), (b2_t, b2),
                         (iota_t, iota), (eye1_t, eye1), (eye2_t, eye2),
                         (dcol_t, dcols), (wcol_t, wcols), (idx_t, idx)):
                nc.sync.dma_start(t[:], d[:])

            def layer(tagp, src_dram, elem, agg_sb):
                with (
                    tc.tile_pool(name=f"{tagp}g", bufs=2) as gpool,
                    tc.tile_pool(name=f"{tagp}m", bufs=6) as mpool,
                    tc.tile_pool(name=f"{tagp}a", bufs=2,
                                 space=bass.MemorySpace.PSUM) as apool,
                ):
                    for (s, g2, q0, nch) in windows:
                        g = gpool.tile([128, max_ch, elem], BF16, tag="g")
                        r0 = s * slab_rows
                        nc.gpsimd.dma_gather(
                            g[:, :nch, :],
                            src_dram[r0:min(r0 + slab_rows, n), :],
                            idx_t[:, q0 * 8:(q0 + nch) * 8],
                            num_idxs=nch * 128,
                            num_idxs_reg=nch * 128,
                            elem_size=elem,
                            single_packet=False,
                        )
                        agg_ps = apool.tile([128, grp_cols], F32, tag="aps")
                        for i in range(nch):
                            q = q0 + i
                            mask = mpool.tile([128, grp_cols], BF16, tag="m")
                            nc.vector.tensor_scalar(
                                mask[:], iota_t[:],
                                dcol_t[:, q:q + 1], wcol_t[:, q:q + 1],
                                op0=mybir.AluOpType.is_equal,
                                op1=mybir.AluOpType.mult,
                            )
                            nc.tensor.matmul(
                                agg_ps[:], g[:, i, :], mask[:],
                                start=(i == 0), stop=(i == nch - 1),
                            )
                        cs = slice(g2 * grp_cols, (g2 + 1) * grp_cols)
                        nc.vector.tensor_add(
                            agg_sb[:, cs], agg_sb[:, cs], agg_ps[:])

            # ---------------- layer 1 ----------------
            with tc.tile_pool(name="acc1", bufs=1) as acc1:
                agg1 = acc1.tile([128, ncols], F32)
                nc.vector.memset(agg1[:], 0.0)
                layer("l1", xg, feat, agg1)

                with tc.tile_pool(name="p1s", bufs=3) as spool, \
                     tc.tile_pool(name="p1p", bufs=2,
                                  space=bass.MemorySpace.PSUM) as mmpool, \
                     tc.tile_pool(name="p1q", bufs=2,
                                  space=bass.MemorySpace.PSUM) as ppool, \
                     tc.tile_pool(name="p1t", bufs=2,
                                  space=bass.MemorySpace.PSUM) as tpool:
                    for b in range(nblk):
                        rows = min(128, npc - b * 128)
                        co = slice(b * 128, b * 128 + 128)
                        aggb = spool.tile([feat, 128], BF16, tag="aggb")
                        nc.scalar.copy(aggb[:], agg1[:, co])
                        mm1 = mmpool.tile([hid, 128], F32, tag="mm1")
                        nc.tensor.matmul(mm1[:], w1_t[:], aggb[:],
                                         start=True, stop=True)
                        h_t = spool.tile([hid, 128], BF16, tag="h")
                        nc.scalar.activation(
                            h_t[:], mm1[:],
                            mybir.ActivationFunctionType.Relu, bias=b1_t[:])
                        p1 = ppool.tile([clsp, 128], F32, tag="p1")
                        nc.tensor.matmul(p1[:], w2_t[:], h_t[:],
                                         start=True, stop=True)
                        p1s = spool.tile([clsp, 128], BF16, tag="p1s")
                        nc.scalar.copy(p1s[:], p1[:])
                        pt = tpool.tile([128, clsp], BF16, tag="pt")
                        nc.tensor.transpose(pt[:], p1s[:], eye1_t[:])
                        po = spool.tile([128, clsp], BF16, tag="po")
                        nc.scalar.copy(po[:], pt[:])
                        nc.sync.dma_start(
                            p_slice[b * 128:b * 128 + rows, :], po[:rows, :])

            nc.gpsimd.collective_compute(
                "AllGather",
                mybir.AluOpType.bypass,
                replica_groups=[list(range(NCORES))],
                ins=[p_slice[:]],
                outs=[p_full[:]],
            )

            # ---------------- layer 2 ----------------
            with tc.tile_pool(name="acc2", bufs=1) as acc2:
                agg2 = acc2.tile([128, ncols], F32)
                nc.vector.memset(agg2[:], 0.0)
                layer("l2", p_full, clsp, agg2)

                with tc.tile_pool(name="p2s", bufs=3) as spool2, \
                     tc.tile_pool(name="p2t", bufs=2,
                                  space=bass.MemorySpace.PSUM) as tpool2:
                    for b in range(nblk):
                        rows = min(128, npc - b * 128)
                        co = slice(b * 128, b * 128 + 128)
                        oS = spool2.tile([64, 128], F32, tag="oS")
                        nc.scalar.activation(
                            oS[:], agg2[0:64, co],
                            mybir.ActivationFunctionType.Identity,
                            bias=b2_t[:])
                        ot = tpool2.tile([128, 64], F32, tag="ot")
                        nc.tensor.transpose(ot[:], oS[:], eye2_t[:])
                        oo = spool2.tile([128, 64], F32, tag="oo")
                        nc.scalar.copy(oo[:], ot[:])
                        nc.sync.dma_start(
                            out[b * 128:b * 128 + rows, :],
                            oo[:rows, :cls])

    nc.compile()
    return nc


# --------------------------------------------------------------------------
# Input assembly
# --------------------------------------------------------------------------

def make_in_maps(cfg, per_core, x, W1, b1, W2, b2):
    hid = W1.shape[1]
    cls = W2.shape[1]
    clsp = 128
    grp_cols = cfg["grp_cols"]
    xg = x.astype(ml_dtypes.bfloat16)
    W1b = np.asarray(W1, np.float32).astype(ml_dtypes.bfloat16)
    W2p = np.zeros((hid, clsp), np.float32)
    W2p[:, :cls] = W2
    W2b = W2p.astype(ml_dtypes.bfloat16)
    b2p = np.zeros(64, np.float32)
    b2p[:cls] = b2
    iota = np.broadcast_to(np.arange(grp_cols, dtype=np.float32),
                           (128, grp_cols)).copy()
    eye1 = np.eye(clsp, dtype=np.float32).astype(ml_dtypes.bfloat16)
    eye2 = np.eye(64, dtype=np.float32)
    ins = []
    for c in range(NCORES):
        pc = per_core[c]
        ins.append({
            "xg": xg, "w1": W1b, "b1": np.asarray(b1, np.float32),
            "w2": W2b, "b2": b2p,
            "idx": pc["idx"], "dcols": pc["dcol"], "wcols": pc["wcol"],
            "iota": iota, "eye1": eye1, "eye2": eye2,
        })
    return ins


# --------------------------------------------------------------------------
# Persistent PJRT runner (NEFF + device-resident inputs reused across runs)
# --------------------------------------------------------------------------

class _Runner:
    def __init__(self, nc, in_maps):
        import jax
        import concourse.mybir as mb
        from concourse.bass2jax import (
            _bass_exec_p, install_neuronx_cc_hook, partition_id_tensor,
            shard_map, Mesh, PartitionSpec,
        )

        install_neuronx_cc_hook()
        n_cores = len(in_maps)
        partition_name = (nc.partition_id_tensor.name
                          if nc.partition_id_tensor else None)
        in_names, out_names, out_avals, zero_outs = [], [], [], []
        for alloc in nc.m.functions[0].allocations:
            if not isinstance(alloc, mb.MemoryLocationSet):
                continue
            name = alloc.memorylocations[0].name
            if alloc.kind == "ExternalInput":
                if name != partition_name:
                    in_names.append(name)
            elif alloc.kind == "ExternalOutput":
                out_names.append(name)
                shape = tuple(alloc.tensor_shape)
                dtype = mb.dt.np(alloc.dtype)
                out_avals.append(jax.core.ShapedArray(shape, dtype))
                zero_outs.append(np.zeros(shape, dtype))
        n_params = len(in_names)
        all_names = in_names + out_names
        if partition_name is not None:
            all_names.append(partition_name)

        def _body(*args):
            operands = list(args)
            if partition_name is not None:
                operands.append(partition_id_tensor())
            return tuple(_bass_exec_p.bind(
                *operands,
                out_avals=tuple(out_avals),
                in_names=tuple(all_names),
                out_names=tuple(out_names),
                lowering_input_output_aliases=(),
                sim_require_finite=True,
                sim_require_nnan=True,
                nc=nc,
            ))

        devices = jax.devices()[:n_cores]
        mesh = Mesh(np.asarray(devices), ("core",))
        nin = n_params + len(out_names)
        self._fn = jax.jit(
            shard_map(_body, mesh=mesh,
                      in_specs=(PartitionSpec("core"),) * nin,
                      out_specs=(PartitionSpec("core"),) * len(out_names),
                      check_rep=False),
            keep_unused=True,
        )
        concat_in = [
            np.concatenate([np.asarray(in_maps[c][k]) for c in range(n_cores)],
                           axis=0)
            for k in in_names
        ]
        concat_zeros = [
            np.zeros((n_cores * z.shape[0], *z.shape[1:]), z.dtype)
            for z in zero_outs
        ]
        self._dev_in = [jax.device_put(a) for a in concat_in + concat_zeros]
        self.out_names = out_names
        self.out_avals = out_avals
        self.n_cores = n_cores

    def run(self):
        outs = self._fn(*self._dev_in)
        for o in outs:
            o.block_until_ready()
        return outs

    def results(self):
        outs = self.run()
        return [
            {
                name: np.asarray(outs[i]).reshape(
                    self.n_cores, *self.out_avals[i].shape)[c]
                for i, name in enumerate(self.out_names)
            }
            for c in range(self.n_cores)
        ]


_CACHE = {}
_RUNNER = {}


def _prep_all(x, W1, b1, W2, b2, src, dst):
    x = np.asarray(x, np.float32)
    src = np.asarray(src, np.int32)
    dst = np.asarray(dst, np.int32)
    n, feat = x.shape
    hid = np.asarray(W1).shape[1]
    cls = np.asarray(W2).shape[1]
    cfg, per_core = preprocess(src, dst, n)
    key = (n, feat, hid, cls, cfg["nchunk"])
    if key not in _CACHE:
        _CACHE[key] = build_program(cfg, feat, hid, cls)
    nc = _CACHE[key]
    ins = make_in_maps(cfg, per_core, x, np.asarray(W1, np.float32),
                       np.asarray(b1, np.float32), np.asarray(W2, np.float32),
                       np.asarray(b2, np.float32))
    return key, cfg, nc, ins


def _get_runner(key, nc, ins):
    if key not in _RUNNER:
        _RUNNER[key] = _Runner(nc, ins)
    return _RUNNER[key]


def time_kernel(inputs, iters=10):
    """Wall-clock the on-device execution (inputs device-resident)."""
    import time
    key, cfg, nc, ins = _prep_all(**inputs)
    r = _get_runner(key, nc, ins)
    r.run()  # warm-up (triggers NEFF compile/load)
    times = []
    for _ in range(iters):
        t0 = time.perf_counter()
        r.run()
        times.append(time.perf_counter() - t0)
    times.sort()
    return times[len(times) // 4] * 1e9   # lower-quartile wall time


def kernel(x, W1, b1, W2, b2, src, dst):
    n = np.asarray(x).shape[0]
    key, cfg, nc, ins = _prep_all(x, W1, b1, W2, b2, src, dst)
    r = _get_runner(key, nc, ins)
    res = r.results()
    return np.concatenate([res[c]["out"] for c in range(NCORES)],
                          axis=0)[:n]
